# revision 23
# baseline (speedup 1.0000x reference)
"""Causal multi-head attention on 8 Trainium2 NeuronCores.

Problem: B=2, H=16, S=2048, D=128 fp32.
  out = softmax(mask(Q K^T) / sqrt(D)) V   per (batch, head)

Sharding: the 32 (batch*head) pairs are split 4-per-core across 8 cores.
Each core computes full causal attention for its 4 heads independently.

Device-side formulation (per head), everything "transposed" so no on-chip
transposes are needed:
  - Host ships Q^T, K^T as [D=128, S] (d-major) bf16 and V as [S, D] bf16.
  - scores^T block [k=128, q=512] = matmul(lhsT=K^T tile, rhs=Q^T chunk) bf16
    into PSUM; causal column shrink on all diagonal-band tiles.
  - P^T = exp(scores^T / sqrt(D)) -> bf16. Two producers share the work:
    ScalarE ACTIVATE (exact, ~(N+352)/1.2 ns) and, for a tunable subset of
    below-diagonal pairs, VectorE via the Schraudolph bit trick
    (round(x*A + B) as int16 IS the bf16 encoding of ~2^(x*log2e), ~3% max
    rel err -- harmless on long rows where errors average out). ScalarE is
    the critical path; the offload buys back its oversubscription.
  - causal masking: the 128x128 diagonal blocks get an additive -1e9 before
    exp (one strided DVE op covers both blocks of a pair). The below-lo
    garbage columns of band tiles are memset to -1e9 so exp makes them 0.0,
    which lets the band denominator use pre-added pairs.
  - PV: out^T [d,q] += matmul(lhsT=V tile [k,d], rhs=P^T) bf16.
  - denominator += matmul(lhsT=ones, rhs=P^T or VectorE pre-added pair/quad
    sums) -- row-broadcast trick.
  - out = out^T * reciprocal(denom) on VectorE -> bf16, DMA out as [D, S];
    host transposes back.
  - the last head processes chunk 0 (4 k-tiles) last so the post-exp tail
    (PV + normalize + DMA of the final chunk) is short.
"""

import numpy as np

B, H, S, D = 2, 16, 2048, 128
N_CORES = 8
HEADS_PER_CORE = (B * H) // N_CORES  # 4
SCALE = 1.0 / float(D) ** 0.5

P = 128          # partition dim / k-tile size
QC = 512         # q chunk width (moving dim; one PSUM bank of fp32)
LA = 2           # score-group lookahead (software pipeline depth)
# Schraudolph offload: below-pair p is computed on VectorE when
# (pair_counter % OFF_MOD) in OFF_PHASES
OFF_MOD = 8
OFF_PHASES = (1, 3, 6)
SCH_A = SCALE * np.log2(np.e) * 128.0
SCH_B = (127.0 - 0.057745) * 128.0
BAND_DEN_PAIR = True  # zero garbage cols, pre-add band pairs for the denom


def build_module(n_heads=HEADS_PER_CORE, s=S):
    """Per-core Bass module. Inputs qT,kT: [n_heads, n_ch, D, QC] bf16,
    v: [n_heads, n_ch, P, 4, P] bf16; output outT: [n_heads, n_ch, D, QC]
    bf16."""
    import concourse.mybir as mybir
    import concourse.tile as tile
    from concourse import bacc
    import concourse.bass as _bass
    from contextlib import ExitStack

    f32 = mybir.dt.float32
    bf16 = mybir.dt.bfloat16
    i16 = mybir.dt.int16
    n_qc = s // QC
    kt_per_qc = QC // P

    nc = bacc.Bacc("TRN2", target_bir_lowering=False, debug=False)

    n_ch = s // QC
    qT = nc.dram_tensor("qT", [n_heads, n_ch, P, QC], bf16, kind="ExternalInput").ap()
    kT = nc.dram_tensor("kT", [n_heads, n_ch, P, QC], bf16, kind="ExternalInput").ap()
    v = nc.dram_tensor("v", [n_heads, n_ch, P, QC // P, P], bf16, kind="ExternalInput").ap()
    outT = nc.dram_tensor("outT", [n_heads, n_ch, P, QC], bf16, kind="ExternalOutput").ap()

    with tile.TileContext(nc) as tc, ExitStack() as ctx:
        const_pool = ctx.enter_context(tc.tile_pool(name="const", bufs=1))
        io_depth = n_ch * min(n_heads, 2)
        q_pool = ctx.enter_context(tc.tile_pool(name="q", bufs=io_depth))
        k_pool = ctx.enter_context(tc.tile_pool(name="k", bufs=io_depth))
        v_pool = ctx.enter_context(tc.tile_pool(name="v", bufs=io_depth))
        pc_pool = ctx.enter_context(tc.tile_pool(name="pc", bufs=2))
        o_pool = ctx.enter_context(tc.tile_pool(name="o", bufs=4))
        s_psum = ctx.enter_context(tc.tile_pool(name="spsum", bufs=3, space="PSUM"))
        o_psum = ctx.enter_context(tc.tile_pool(name="opsum", bufs=1, space="PSUM"))
        d_psum = ctx.enter_context(tc.tile_pool(name="dpsum", bufs=1, space="PSUM"))

        # head 0 / chunk 0's inputs first: nothing upstream delays the first
        # QK matmul. k split in halves so the first band pair starts sooner.
        k00 = k_pool.tile([P, QC], bf16, tag="k", name="k00")
        nc.sync.dma_start(out=k00[:, 0:QC // 2], in_=kT[0, 0, :, 0:QC // 2])
        q00 = q_pool.tile([P, QC], bf16, tag="q", name="q00")
        nc.gpsimd.dma_start(out=q00[:], in_=qT[0, 0])
        nc.sync.dma_start(out=k00[:, QC // 2:], in_=kT[0, 0, :, QC // 2:])
        v00 = v_pool.tile([P, QC // P, P], bf16, tag="v", name="v00")
        nc.sync.dma_start(out=v00[:], in_=v[0, 0])

        ones_bf = const_pool.tile([P, P], bf16)
        nc.gpsimd.memset(ones_bf[:], 1.0)
        # additive causal mask for the 128x128 diagonal block
        mask_add = const_pool.tile([P, P], f32)
        nc.gpsimd.memset(mask_add[:], 0.0)
        nc.gpsimd.affine_select(
            out=mask_add[:],
            in_=mask_add[:],
            compare_op=mybir.AluOpType.is_ge,
            fill=-1e9,
            base=0,
            channel_multiplier=-1,
            pattern=[[1, P]],
        )
        # warm the exp table set off the critical path
        warm = const_pool.tile([1, 1], f32)
        nc.vector.memset(warm[:], 0.0)
        nc.scalar.activation(warm[:], warm[:],
                             mybir.ActivationFunctionType.Exp)

        pair_ctr = [0]

        for h in range(n_heads):
            qs_c, ks_c, vs_c = [], [], []
            for cch in range(n_ch):
                if h == 0 and cch == 0:
                    ks_c.append(k00)
                    qs_c.append(q00)
                    vs_c.append(v00)
                    continue
                kc = k_pool.tile([P, QC], bf16, tag="k")
                nc.sync.dma_start(out=kc[:], in_=kT[h, cch])
                ks_c.append(kc)
                qc_t = q_pool.tile([P, QC], bf16, tag="q")
                nc.gpsimd.dma_start(out=qc_t[:], in_=qT[h, cch])
                qs_c.append(qc_t)
                vc = v_pool.tile([P, QC // P, P], bf16, tag="v")
                (nc.gpsimd if cch % 2 else nc.sync).dma_start(
                    out=vc[:], in_=v[h, cch]
                )
                vs_c.append(vc)

            chunk_order = list(range(n_qc))
            if h == n_heads - 1:
                chunk_order = chunk_order[1:] + [0]  # short tail

            for qc in chunk_order:
                out_ps = o_psum.tile([P, QC], f32, tag="o")
                den_ps = d_psum.tile([P, QC], f32, tag="d")
                nkt = kt_per_qc * (qc + 1)
                n_below = nkt - kt_per_qc
                q_sl = qs_c[qc][:]
                # all of this chunk's P values (below + band), bf16
                pcb = pc_pool.tile([P, 16 * QC], bf16, tag="pcb")

                groups = []
                for t in range(n_below // 2):
                    off = pair_ctr[0] % OFF_MOD in OFF_PHASES
                    pair_ctr[0] += 1
                    groups.append(("below", t, [2 * t, 2 * t + 1], off))
                groups.append(("band", 0, [n_below, n_below + 1], False))
                groups.append(("band", 1, [n_below + 2, n_below + 3], False))

                def k_sl(kt):
                    return ks_c[kt // kt_per_qc][
                        :, (kt % kt_per_qc) * P:(kt % kt_per_qc + 1) * P]

                def emit_qk_exp(gi, qc=qc, groups=groups, q_sl=q_sl, pcb=pcb,
                                k_sl=k_sl):
                    kind, idx, gkts, off = groups[gi]
                    s_ps = s_psum.tile([P, 2 * QC], f32, tag="s")
                    po = gkts[0] * QC  # P destination offset (flat cols)
                    if kind == "below":
                        for i, kt in enumerate(gkts):
                            nc.tensor.matmul(
                                s_ps[:, i * QC:(i + 1) * QC],
                                lhsT=k_sl(kt), rhs=q_sl[:],
                                start=True, stop=True,
                            )
                        if off:
                            # Schraudolph: int16(round(x*A + B)) bits are the
                            # bf16 encoding of ~exp(x*SCALE)
                            nc.vector.tensor_scalar(
                                pcb[:, po:po + 2 * QC].bitcast(i16),
                                s_ps[:],
                                SCH_A, SCH_B,
                                mybir.AluOpType.mult, mybir.AluOpType.add,
                            )
                        else:
                            nc.scalar.activation(
                                pcb[:, po:po + 2 * QC], s_ps[:],
                                mybir.ActivationFunctionType.Exp,
                                scale=SCALE,
                            )
                    else:
                        for i, kt in enumerate(gkts):
                            c = kt * P - qc * QC
                            lo = max(c, 0)
                            nc.tensor.matmul(
                                s_ps[:, i * QC + lo:(i + 1) * QC],
                                lhsT=k_sl(kt), rhs=q_sl[:, lo:QC],
                                start=True, stop=True,
                            )
                        c0 = gkts[0] * P - qc * QC
                        # exp the raw scores (max ~exp(8.4)=4.4e3, bf16-safe);
                        # causal masking happens post-exp in SBUF on GpSimd
                        nc.scalar.activation(
                            pcb[:, po + c0:po + 2 * QC],
                            s_ps[:, c0:2 * QC],
                            mybir.ActivationFunctionType.Exp,
                            scale=SCALE,
                        )

                for gi in range(min(LA + 1, len(groups))):
                    emit_qk_exp(gi)

                first_pv = [True]
                first_den = [True]
                last_kt = nkt - 1
                n_pairs_below = n_below // 2
                # denominator plan for below pairs: merge adjacent pairs
                # (quads) via one extra VectorE add
                den_rhs = [None] * max(n_pairs_below, 1)

                def p_sl(kt, lo=0):
                    return pcb[:, kt * QC + lo:(kt + 1) * QC]

                for gi, (kind, idx, gkts, off) in enumerate(groups):
                    if kind == "below":
                        t = idx
                        # PV: two solo bf16 matmuls
                        for i, kt in enumerate(gkts):
                            nc.tensor.matmul(
                                out_ps[:],
                                lhsT=vs_c[kt // kt_per_qc][:, kt % kt_per_qc, :],
                                rhs=p_sl(kt),
                                start=first_pv[0], stop=False,
                            )
                            first_pv[0] = False
                        # den: every 2nd pair, reduce the quad (4 k-tiles)
                        # with two VectorE adds and one matmul. n_below is a
                        # multiple of 4 so quads always complete.
                        if t % 2 == 1:
                            b = gkts[0] - 2  # first tile of the quad
                            # strided double add: [t0+t1 | t2+t3] in one op
                            pa = pcb[:, b * QC:]
                            va = _bass.AP(
                                pa.tensor, pa.offset,
                                [pa.ap[0], [2 * QC, 2], [1, QC]])
                            pb = pcb[:, b * QC + QC:]
                            vb = _bass.AP(
                                pb.tensor, pb.offset,
                                [pb.ap[0], [2 * QC, 2], [1, QC]])
                            p01d = o_pool.tile([P, 2 * QC], bf16, tag="p01")
                            vo = _bass.AP(
                                p01d.tensor, p01d.offset,
                                [p01d.ap[0], [QC, 2], [1, QC]])
                            nc.vector.tensor_add(vo, va, vb)
                            p03 = o_pool.tile([P, QC], bf16, tag="p03")
                            nc.vector.tensor_add(
                                p03[:], p01d[:, 0:QC], p01d[:, QC:2 * QC])
                            nc.tensor.matmul(
                                den_ps[:],
                                lhsT=ones_bf[:], rhs=p03[:],
                                start=first_den[0], stop=False,
                            )
                            first_den[0] = False
                    else:
                        c0 = gkts[0] * P - qc * QC
                        if idx == 0:
                            # batched post-exp causal masking for all 4 band
                            # tiles: one strided affine_select zeroes the
                            # strictly-below-diagonal of the four 128x128
                            # diagonal blocks (QC+P apart), one strided
                            # memset zeroes the exp'd garbage columns of
                            # tiles b1 and b3 (for clean den pair sums)
                            nb = nkt - kt_per_qc
                            d0 = pcb[:, nb * QC:]
                            dview = _bass.AP(
                                d0.tensor, d0.offset,
                                [d0.ap[0], [QC + P, 4], [1, P]])
                            nc.gpsimd.affine_select(
                                out=dview, in_=dview,
                                compare_op=mybir.AluOpType.is_ge,
                                fill=0.0, base=0,
                                channel_multiplier=-1,
                                pattern=[[0, 4], [1, P]],
                            )
                            if BAND_DEN_PAIR:
                                g0 = pcb[:, (nb + 1) * QC:]
                                gview = _bass.AP(
                                    g0.tensor, g0.offset,
                                    [g0.ap[0], [2 * QC + 2 * P, 2], [1, P]])
                                nc.vector.memset(gview, 0.0)
                        # PV: solo bf16 matmuls with causal shrink
                        for i, kt in enumerate(gkts):
                            c = kt * P - qc * QC
                            nc.tensor.matmul(
                                out_ps[:, c:QC],
                                lhsT=vs_c[kt // kt_per_qc][:, kt % kt_per_qc, :],
                                rhs=p_sl(kt, c),
                                start=first_pv[0], stop=(kt == last_kt),
                            )
                            first_pv[0] = False
                        if BAND_DEN_PAIR:
                            # garbage cols were zeroed: pre-add the band pair
                            pb2 = o_pool.tile([P, QC], bf16, tag="p01")
                            nc.vector.tensor_add(
                                pb2[:, c0:QC],
                                p_sl(gkts[0], c0), p_sl(gkts[1], c0))
                            nc.tensor.matmul(
                                den_ps[:, c0:QC],
                                lhsT=ones_bf[:], rhs=pb2[:, c0:QC],
                                start=first_den[0],
                                stop=(gkts[1] == last_kt),
                            )
                            first_den[0] = False
                        else:
                            for i, kt in enumerate(gkts):
                                c = kt * P - qc * QC
                                nc.tensor.matmul(
                                    den_ps[:, c:QC],
                                    lhsT=ones_bf[:], rhs=p_sl(kt, c),
                                    start=first_den[0], stop=(kt == last_kt),
                                )
                                first_den[0] = False
                    if gi + LA + 1 < len(groups):
                        emit_qk_exp(gi + LA + 1)

                recip = o_pool.tile([P, QC], f32, tag="r")
                nc.vector.reciprocal_approx_fast(out=recip[:], in_=den_ps[:])
                o_sb = o_pool.tile([P, QC], bf16, tag="os")
                nc.vector.tensor_mul(o_sb[:], out_ps[:], recip[:])
                nc.sync.dma_start(out=outT[h, qc], in_=o_sb[:])

    nc.compile()
    return nc


def pack_shard(qh, kh, vh):
    """Pack per-core arrays [n_heads, s, D] into the kernel's DRAM layouts."""
    import ml_dtypes
    nh, s, _ = qh.shape
    n_ch = s // QC
    qT = np.ascontiguousarray(
        qh.transpose(0, 2, 1).reshape(nh, D, n_ch, QC).transpose(0, 2, 1, 3)
    ).astype(ml_dtypes.bfloat16)
    kT = np.ascontiguousarray(
        kh.transpose(0, 2, 1).reshape(nh, D, n_ch, QC).transpose(0, 2, 1, 3)
    ).astype(ml_dtypes.bfloat16)
    v5 = np.ascontiguousarray(
        vh.reshape(nh, n_ch, QC // P, P, D).transpose(0, 1, 3, 2, 4)
    ).astype(ml_dtypes.bfloat16)
    return {"qT": qT, "kT": kT, "v": v5}


def unpack_out(outT):
    """outT [nh, n_ch, D, QC] bf16 -> [nh, s, D] f32."""
    nh, n_ch, _, _ = outT.shape
    o = outT.astype(np.float32).transpose(0, 2, 1, 3).reshape(nh, D, n_ch * QC)
    return o.transpose(0, 2, 1)


_NC_CACHE = {}


def _get_module():
    key = (HEADS_PER_CORE, S)
    if key not in _NC_CACHE:
        _NC_CACHE[key] = build_module(*key)
    return _NC_CACHE[key]


def kernel(q, k, v):
    from concourse.bass_utils import run_bass_kernel_spmd

    q = np.asarray(q, dtype=np.float32)
    k = np.asarray(k, dtype=np.float32)
    v = np.asarray(v, dtype=np.float32)

    qf = q.reshape(B * H, S, D)
    kf = k.reshape(B * H, S, D)
    vf = v.reshape(B * H, S, D)
    hpc = HEADS_PER_CORE
    in_maps = [
        pack_shard(
            qf[c * hpc:(c + 1) * hpc],
            kf[c * hpc:(c + 1) * hpc],
            vf[c * hpc:(c + 1) * hpc],
        )
        for c in range(N_CORES)
    ]

    nc = _get_module()
    res = run_bass_kernel_spmd(nc, in_maps, core_ids=list(range(N_CORES)))
    out = np.concatenate(
        [unpack_out(r["outT"]) for r in res.results], axis=0
    ).reshape(B, H, S, D)
    return np.ascontiguousarray(out.astype(np.float32))


# revision 26
# speedup vs baseline: 1.1632x; 1.1632x over previous
"""Causal multi-head attention on 8 Trainium2 NeuronCores.

Problem: B=2, H=16, S=2048, D=128 fp32.
  out = softmax(mask(Q K^T) / sqrt(D)) V   per (batch, head)

Sharding: the 32 (batch*head) pairs are split 4-per-core across 8 cores.
Each core computes full causal attention for its 4 heads independently.

Device-side formulation (per head), everything "transposed" so no on-chip
transposes are needed:
  - Host ships Q^T, K^T as [D=128, S] (d-major) bf16 and V as [S, D] bf16.
  - scores^T block [k=128, q=512] = matmul(lhsT=K^T tile, rhs=Q^T chunk) bf16
    into PSUM; causal column shrink on all diagonal-band tiles.
  - P^T = exp(scores^T / sqrt(D)) -> bf16. Two producers share the work:
    ScalarE ACTIVATE (exact, ~(N+352)/1.2 ns) and, for a tunable subset of
    below-diagonal pairs, VectorE via the Schraudolph bit trick
    (round(x*A + B) as int16 IS the bf16 encoding of ~2^(x*log2e), ~3% max
    rel err -- harmless on long rows where errors average out). ScalarE is
    the critical path; the offload buys back its oversubscription.
  - causal masking: the 128x128 diagonal blocks get an additive -1e9 before
    exp (one strided DVE op covers both blocks of a pair). The below-lo
    garbage columns of band tiles are memset to -1e9 so exp makes them 0.0,
    which lets the band denominator use pre-added pairs.
  - PV: out^T [d,q] += matmul(lhsT=V tile [k,d], rhs=P^T) bf16.
  - denominator += matmul(lhsT=ones, rhs=P^T or VectorE pre-added pair/quad
    sums) -- row-broadcast trick.
  - out = out^T * reciprocal(denom) on VectorE -> bf16, DMA out as [D, S];
    host transposes back.
  - the last head processes chunk 0 (4 k-tiles) last so the post-exp tail
    (PV + normalize + DMA of the final chunk) is short.
"""

import numpy as np

B, H, S, D = 2, 16, 2048, 128
N_CORES = 8
HEADS_PER_CORE = (B * H) // N_CORES  # 4
SCALE = 1.0 / float(D) ** 0.5

P = 128          # partition dim / k-tile size
QC = 512         # q chunk width (moving dim; one PSUM bank of fp32)
LA = 2           # score-group lookahead (software pipeline depth)
# Schraudolph offload: below-pair p is computed on VectorE when
# (pair_counter % OFF_MOD) in OFF_PHASES
OFF_MOD = 7
OFF_PHASES = (1, 4)
SCH_A = SCALE * np.log2(np.e) * 128.0
SCH_B = (127.0 - 0.057745) * 128.0
BAND_DEN_PAIR = True  # zero garbage cols, pre-add band pairs for the denom


def build_module(n_heads=HEADS_PER_CORE, s=S):
    """Per-core Bass module. Inputs qT,kT: [n_heads, n_ch, D, QC] bf16,
    v: [n_heads, n_ch, P, 4, P] bf16; output outT: [n_heads, n_ch, D, QC]
    bf16."""
    import concourse.mybir as mybir
    import concourse.tile as tile
    from concourse import bacc
    import concourse.bass as _bass
    from contextlib import ExitStack

    f32 = mybir.dt.float32
    bf16 = mybir.dt.bfloat16
    i16 = mybir.dt.int16
    n_qc = s // QC
    kt_per_qc = QC // P

    nc = bacc.Bacc("TRN2", target_bir_lowering=False, debug=False)

    n_ch = s // QC
    qT = nc.dram_tensor("qT", [n_heads, n_ch, P, QC], bf16, kind="ExternalInput").ap()
    kT = nc.dram_tensor("kT", [n_heads, n_ch, P, QC], bf16, kind="ExternalInput").ap()
    v = nc.dram_tensor("v", [n_heads, n_ch, P, QC // P, P], bf16, kind="ExternalInput").ap()
    outT = nc.dram_tensor("outT", [n_heads, n_ch, P, QC], bf16, kind="ExternalOutput").ap()

    with tile.TileContext(nc) as tc, ExitStack() as ctx:
        const_pool = ctx.enter_context(tc.tile_pool(name="const", bufs=1))
        io_depth = n_ch * min(n_heads, 2)
        q_pool = ctx.enter_context(tc.tile_pool(name="q", bufs=io_depth))
        k_pool = ctx.enter_context(tc.tile_pool(name="k", bufs=io_depth))
        v_pool = ctx.enter_context(tc.tile_pool(name="v", bufs=io_depth))
        pc_pool = ctx.enter_context(tc.tile_pool(name="pc", bufs=2))
        o_pool = ctx.enter_context(tc.tile_pool(name="o", bufs=4))
        s_psum = ctx.enter_context(tc.tile_pool(name="spsum", bufs=3, space="PSUM"))
        o_psum = ctx.enter_context(tc.tile_pool(name="opsum", bufs=1, space="PSUM"))
        d_psum = ctx.enter_context(tc.tile_pool(name="dpsum", bufs=1, space="PSUM"))

        # head 0 / chunk 0's inputs first: nothing upstream delays the first
        # QK matmul. k split in halves so the first band pair starts sooner.
        k00 = k_pool.tile([P, QC], bf16, tag="k", name="k00")
        nc.sync.dma_start(out=k00[:, 0:QC // 2], in_=kT[0, 0, :, 0:QC // 2])
        q00 = q_pool.tile([P, QC], bf16, tag="q", name="q00")
        nc.gpsimd.dma_start(out=q00[:], in_=qT[0, 0])
        nc.sync.dma_start(out=k00[:, QC // 2:], in_=kT[0, 0, :, QC // 2:])
        v00 = v_pool.tile([P, QC // P, P], bf16, tag="v", name="v00")
        nc.sync.dma_start(out=v00[:], in_=v[0, 0])

        ones_bf = const_pool.tile([P, P], bf16)
        nc.gpsimd.memset(ones_bf[:], 1.0)
        # additive causal mask for the 128x128 diagonal block
        mask_add = const_pool.tile([P, P], f32)
        nc.gpsimd.memset(mask_add[:], 0.0)
        nc.gpsimd.affine_select(
            out=mask_add[:],
            in_=mask_add[:],
            compare_op=mybir.AluOpType.is_ge,
            fill=-1e9,
            base=0,
            channel_multiplier=-1,
            pattern=[[1, P]],
        )
        # warm the exp table set off the critical path
        warm = const_pool.tile([1, 1], f32)
        nc.vector.memset(warm[:], 0.0)
        nc.scalar.activation(warm[:], warm[:],
                             mybir.ActivationFunctionType.Exp)

        pair_ctr = [0]

        for h in range(n_heads):
            qs_c, ks_c, vs_c = [], [], []
            for cch in range(n_ch):
                if h == 0 and cch == 0:
                    ks_c.append(k00)
                    qs_c.append(q00)
                    vs_c.append(v00)
                    continue
                kc = k_pool.tile([P, QC], bf16, tag="k")
                nc.sync.dma_start(out=kc[:], in_=kT[h, cch])
                ks_c.append(kc)
                qc_t = q_pool.tile([P, QC], bf16, tag="q")
                nc.gpsimd.dma_start(out=qc_t[:], in_=qT[h, cch])
                qs_c.append(qc_t)
                vc = v_pool.tile([P, QC // P, P], bf16, tag="v")
                (nc.gpsimd if cch % 2 else nc.sync).dma_start(
                    out=vc[:], in_=v[h, cch]
                )
                vs_c.append(vc)

            chunk_order = list(range(n_qc))
            if h == n_heads - 1:
                chunk_order = chunk_order[1:] + [0]  # short tail

            for qc in chunk_order:
                out_ps = o_psum.tile([P, QC], f32, tag="o")
                den_ps = d_psum.tile([P, QC], f32, tag="d")
                nkt = kt_per_qc * (qc + 1)
                n_below = nkt - kt_per_qc
                q_sl = qs_c[qc][:]
                # all of this chunk's P values (below + band), bf16
                pcb = pc_pool.tile([P, 16 * QC], bf16, tag="pcb")

                groups = []
                for t in range(n_below // 2):
                    off = pair_ctr[0] % OFF_MOD in OFF_PHASES
                    pair_ctr[0] += 1
                    groups.append(("below", t, [2 * t, 2 * t + 1], off))
                groups.append(("band", 0, [n_below, n_below + 1], False))
                groups.append(("band", 1, [n_below + 2, n_below + 3], False))

                def k_sl(kt):
                    return ks_c[kt // kt_per_qc][
                        :, (kt % kt_per_qc) * P:(kt % kt_per_qc + 1) * P]

                def emit_qk_exp(gi, qc=qc, groups=groups, q_sl=q_sl, pcb=pcb,
                                k_sl=k_sl):
                    kind, idx, gkts, off = groups[gi]
                    s_ps = s_psum.tile([P, 2 * QC], f32, tag="s")
                    po = gkts[0] * QC  # P destination offset (flat cols)
                    if kind == "below":
                        for i, kt in enumerate(gkts):
                            nc.tensor.matmul(
                                s_ps[:, i * QC:(i + 1) * QC],
                                lhsT=k_sl(kt), rhs=q_sl[:],
                                start=True, stop=True,
                            )
                        if off:
                            # Schraudolph: int16(round(x*A + B)) bits are the
                            # bf16 encoding of ~exp(x*SCALE)
                            nc.vector.tensor_scalar(
                                pcb[:, po:po + 2 * QC].bitcast(i16),
                                s_ps[:],
                                SCH_A, SCH_B,
                                mybir.AluOpType.mult, mybir.AluOpType.add,
                            )
                        else:
                            nc.scalar.activation(
                                pcb[:, po:po + 2 * QC], s_ps[:],
                                mybir.ActivationFunctionType.Exp,
                                scale=SCALE,
                            )
                    else:
                        for i, kt in enumerate(gkts):
                            c = kt * P - qc * QC
                            lo = max(c, 0)
                            nc.tensor.matmul(
                                s_ps[:, i * QC + lo:(i + 1) * QC],
                                lhsT=k_sl(kt), rhs=q_sl[:, lo:QC],
                                start=True, stop=True,
                            )
                        c0 = gkts[0] * P - qc * QC
                        # exp the raw scores (max ~exp(8.4)=4.4e3, bf16-safe);
                        # causal masking happens post-exp in SBUF on GpSimd
                        nc.scalar.activation(
                            pcb[:, po + c0:po + 2 * QC],
                            s_ps[:, c0:2 * QC],
                            mybir.ActivationFunctionType.Exp,
                            scale=SCALE,
                        )

                for gi in range(min(LA + 1, len(groups))):
                    emit_qk_exp(gi)

                first_pv = [True]
                first_den = [True]
                last_kt = nkt - 1
                n_pairs_below = n_below // 2
                # denominator plan for below pairs: merge adjacent pairs
                # (quads) via one extra VectorE add
                den_rhs = [None] * max(n_pairs_below, 1)

                def p_sl(kt, lo=0):
                    return pcb[:, kt * QC + lo:(kt + 1) * QC]

                for gi, (kind, idx, gkts, off) in enumerate(groups):
                    if kind == "below":
                        t = idx
                        # PV: two solo bf16 matmuls
                        for i, kt in enumerate(gkts):
                            nc.tensor.matmul(
                                out_ps[:],
                                lhsT=vs_c[kt // kt_per_qc][:, kt % kt_per_qc, :],
                                rhs=p_sl(kt),
                                start=first_pv[0], stop=False,
                            )
                            first_pv[0] = False
                        # den: every 2nd pair, reduce the quad (4 k-tiles)
                        # with two VectorE adds and one matmul. n_below is a
                        # multiple of 4 so quads always complete.
                        p01 = o_pool.tile([P, QC], bf16, tag="p01")
                        nc.vector.tensor_add(
                            p01[:], p_sl(gkts[0]), p_sl(gkts[1]))
                        den_rhs[t] = p01
                        if t % 2 == 1:
                            # quad: add two pair-sums, one den matmul / 4
                            # tiles (n_below is a multiple of 4)
                            p03 = o_pool.tile([P, QC], bf16, tag="p03")
                            nc.vector.tensor_add(
                                p03[:], den_rhs[t - 1][:], p01[:])
                            nc.tensor.matmul(
                                den_ps[:],
                                lhsT=ones_bf[:], rhs=p03[:],
                                start=first_den[0], stop=False,
                            )
                            first_den[0] = False
                    else:
                        c0 = gkts[0] * P - qc * QC
                        for i, kt in enumerate(gkts):
                            c = kt * P - qc * QC
                            blk = pcb[:, kt * QC + c:kt * QC + c + P]
                            # zero strictly-below-diagonal of the 128x128
                            # diagonal block post-exp (keep where col >= row)
                            nc.gpsimd.affine_select(
                                out=blk, in_=blk,
                                compare_op=mybir.AluOpType.is_ge,
                                fill=0.0, base=0,
                                channel_multiplier=-1,
                                pattern=[[1, P]],
                            )
                            if BAND_DEN_PAIR and i == 1:
                                # zero the second tile's exp'd garbage cols
                                # (below its shrink point) for clean pair sums
                                nc.gpsimd.memset(
                                    pcb[:, kt * QC + c0:kt * QC + c], 0.0)
                        # PV: solo bf16 matmuls with causal shrink
                        for i, kt in enumerate(gkts):
                            c = kt * P - qc * QC
                            nc.tensor.matmul(
                                out_ps[:, c:QC],
                                lhsT=vs_c[kt // kt_per_qc][:, kt % kt_per_qc, :],
                                rhs=p_sl(kt, c),
                                start=first_pv[0], stop=(kt == last_kt),
                            )
                            first_pv[0] = False
                        if BAND_DEN_PAIR:
                            # garbage cols were zeroed: pre-add the band pair
                            pb2 = o_pool.tile([P, QC], bf16, tag="p01")
                            nc.vector.tensor_add(
                                pb2[:, c0:QC],
                                p_sl(gkts[0], c0), p_sl(gkts[1], c0))
                            nc.tensor.matmul(
                                den_ps[:, c0:QC],
                                lhsT=ones_bf[:], rhs=pb2[:, c0:QC],
                                start=first_den[0],
                                stop=(gkts[1] == last_kt),
                            )
                            first_den[0] = False
                        else:
                            for i, kt in enumerate(gkts):
                                c = kt * P - qc * QC
                                nc.tensor.matmul(
                                    den_ps[:, c:QC],
                                    lhsT=ones_bf[:], rhs=p_sl(kt, c),
                                    start=first_den[0], stop=(kt == last_kt),
                                )
                                first_den[0] = False
                    if gi + LA + 1 < len(groups):
                        emit_qk_exp(gi + LA + 1)

                recip = o_pool.tile([P, QC], f32, tag="r")
                nc.vector.reciprocal_approx_fast(out=recip[:], in_=den_ps[:])
                o_sb = o_pool.tile([P, QC], bf16, tag="os")
                nc.vector.tensor_mul(o_sb[:], out_ps[:], recip[:])
                nc.sync.dma_start(out=outT[h, qc], in_=o_sb[:])

    nc.compile()
    return nc


def pack_shard(qh, kh, vh):
    """Pack per-core arrays [n_heads, s, D] into the kernel's DRAM layouts."""
    import ml_dtypes
    nh, s, _ = qh.shape
    n_ch = s // QC
    qT = np.ascontiguousarray(
        qh.transpose(0, 2, 1).reshape(nh, D, n_ch, QC).transpose(0, 2, 1, 3)
    ).astype(ml_dtypes.bfloat16)
    kT = np.ascontiguousarray(
        kh.transpose(0, 2, 1).reshape(nh, D, n_ch, QC).transpose(0, 2, 1, 3)
    ).astype(ml_dtypes.bfloat16)
    v5 = np.ascontiguousarray(
        vh.reshape(nh, n_ch, QC // P, P, D).transpose(0, 1, 3, 2, 4)
    ).astype(ml_dtypes.bfloat16)
    return {"qT": qT, "kT": kT, "v": v5}


def unpack_out(outT):
    """outT [nh, n_ch, D, QC] bf16 -> [nh, s, D] f32."""
    nh, n_ch, _, _ = outT.shape
    o = outT.astype(np.float32).transpose(0, 2, 1, 3).reshape(nh, D, n_ch * QC)
    return o.transpose(0, 2, 1)


_NC_CACHE = {}


def _get_module():
    key = (HEADS_PER_CORE, S)
    if key not in _NC_CACHE:
        _NC_CACHE[key] = build_module(*key)
    return _NC_CACHE[key]


def kernel(q, k, v):
    from concourse.bass_utils import run_bass_kernel_spmd

    q = np.asarray(q, dtype=np.float32)
    k = np.asarray(k, dtype=np.float32)
    v = np.asarray(v, dtype=np.float32)

    qf = q.reshape(B * H, S, D)
    kf = k.reshape(B * H, S, D)
    vf = v.reshape(B * H, S, D)
    hpc = HEADS_PER_CORE
    in_maps = [
        pack_shard(
            qf[c * hpc:(c + 1) * hpc],
            kf[c * hpc:(c + 1) * hpc],
            vf[c * hpc:(c + 1) * hpc],
        )
        for c in range(N_CORES)
    ]

    nc = _get_module()
    res = run_bass_kernel_spmd(nc, in_maps, core_ids=list(range(N_CORES)))
    out = np.concatenate(
        [unpack_out(r["outT"]) for r in res.results], axis=0
    ).reshape(B, H, S, D)
    return np.ascontiguousarray(out.astype(np.float32))


# revision 27
# speedup vs baseline: 1.1772x; 1.0120x over previous
"""Causal multi-head attention on 8 Trainium2 NeuronCores.

Problem: B=2, H=16, S=2048, D=128 fp32.
  out = softmax(mask(Q K^T) / sqrt(D)) V   per (batch, head)

Sharding: the 32 (batch*head) pairs are split 4-per-core across 8 cores.
Each core computes full causal attention for its 4 heads independently.

Device-side formulation (per head), everything "transposed" so no on-chip
transposes are needed:
  - Host ships Q^T, K^T as [D=128, S] (d-major) bf16 and V as [S, D] bf16.
  - scores^T block [k=128, q=512] = matmul(lhsT=K^T tile, rhs=Q^T chunk) bf16
    into PSUM; causal column shrink on all diagonal-band tiles.
  - P^T = exp(scores^T / sqrt(D)) -> bf16. Two producers share the work:
    ScalarE ACTIVATE (exact, ~(N+352)/1.2 ns) and, for a tunable subset of
    below-diagonal pairs, VectorE via the Schraudolph bit trick
    (round(x*A + B) as int16 IS the bf16 encoding of ~2^(x*log2e), ~3% max
    rel err -- harmless on long rows where errors average out). ScalarE is
    the critical path; the offload buys back its oversubscription.
  - causal masking: the 128x128 diagonal blocks get an additive -1e9 before
    exp (one strided DVE op covers both blocks of a pair). The below-lo
    garbage columns of band tiles are memset to -1e9 so exp makes them 0.0,
    which lets the band denominator use pre-added pairs.
  - PV: out^T [d,q] += matmul(lhsT=V tile [k,d], rhs=P^T) bf16.
  - denominator += matmul(lhsT=ones, rhs=P^T or VectorE pre-added pair/quad
    sums) -- row-broadcast trick.
  - out = out^T * reciprocal(denom) on VectorE -> bf16, DMA out as [D, S];
    host transposes back.
  - the last head processes chunk 0 (4 k-tiles) last so the post-exp tail
    (PV + normalize + DMA of the final chunk) is short.
"""

import numpy as np

B, H, S, D = 2, 16, 2048, 128
N_CORES = 8
HEADS_PER_CORE = (B * H) // N_CORES  # 4
SCALE = 1.0 / float(D) ** 0.5

P = 128          # partition dim / k-tile size
QC = 512         # q chunk width (moving dim; one PSUM bank of fp32)
LA = 2           # score-group lookahead (software pipeline depth)
# Schraudolph offload: below-pair p is computed on VectorE when
# (pair_counter % OFF_MOD) in OFF_PHASES
OFF_MOD = 8
OFF_PHASES = (1, 3, 6)
SCH_A = SCALE * np.log2(np.e) * 128.0
SCH_B = (127.0 - 0.057745) * 128.0
BAND_DEN_PAIR = True  # zero garbage cols, pre-add band pairs for the denom


def build_module(n_heads=HEADS_PER_CORE, s=S):
    """Per-core Bass module. Inputs qT,kT: [n_heads, n_ch, D, QC] bf16,
    v: [n_heads, n_ch, P, 4, P] bf16; output outT: [n_heads, n_ch, D, QC]
    bf16."""
    import concourse.mybir as mybir
    import concourse.tile as tile
    from concourse import bacc
    import concourse.bass as _bass
    from contextlib import ExitStack

    f32 = mybir.dt.float32
    bf16 = mybir.dt.bfloat16
    i16 = mybir.dt.int16
    n_qc = s // QC
    kt_per_qc = QC // P

    nc = bacc.Bacc("TRN2", target_bir_lowering=False, debug=False)

    n_ch = s // QC
    qT = nc.dram_tensor("qT", [n_heads, n_ch, P, QC], bf16, kind="ExternalInput").ap()
    kT = nc.dram_tensor("kT", [n_heads, n_ch, P, QC], bf16, kind="ExternalInput").ap()
    v = nc.dram_tensor("v", [n_heads, n_ch, P, QC // P, P], bf16, kind="ExternalInput").ap()
    outT = nc.dram_tensor("outT", [n_heads, n_ch, P, QC], bf16, kind="ExternalOutput").ap()

    with tile.TileContext(nc) as tc, ExitStack() as ctx:
        const_pool = ctx.enter_context(tc.tile_pool(name="const", bufs=1))
        io_depth = n_ch * min(n_heads, 2)
        q_pool = ctx.enter_context(tc.tile_pool(name="q", bufs=io_depth))
        k_pool = ctx.enter_context(tc.tile_pool(name="k", bufs=io_depth))
        v_pool = ctx.enter_context(tc.tile_pool(name="v", bufs=io_depth))
        pc_pool = ctx.enter_context(tc.tile_pool(name="pc", bufs=2))
        o_pool = ctx.enter_context(tc.tile_pool(name="o", bufs=4))
        s_psum = ctx.enter_context(tc.tile_pool(name="spsum", bufs=3, space="PSUM"))
        o_psum = ctx.enter_context(tc.tile_pool(name="opsum", bufs=1, space="PSUM"))
        d_psum = ctx.enter_context(tc.tile_pool(name="dpsum", bufs=1, space="PSUM"))

        # head 0 / chunk 0's inputs first: nothing upstream delays the first
        # QK matmul. k split in halves so the first band pair starts sooner.
        k00 = k_pool.tile([P, QC], bf16, tag="k", name="k00")
        nc.sync.dma_start(out=k00[:, 0:QC // 2], in_=kT[0, 0, :, 0:QC // 2])
        q00 = q_pool.tile([P, QC], bf16, tag="q", name="q00")
        nc.gpsimd.dma_start(out=q00[:], in_=qT[0, 0])
        nc.sync.dma_start(out=k00[:, QC // 2:], in_=kT[0, 0, :, QC // 2:])
        v00 = v_pool.tile([P, QC // P, P], bf16, tag="v", name="v00")
        nc.sync.dma_start(out=v00[:], in_=v[0, 0])

        ones_bf = const_pool.tile([P, P], bf16)
        nc.gpsimd.memset(ones_bf[:], 1.0)
        # additive causal mask for the 128x128 diagonal block
        mask_add = const_pool.tile([P, P], f32)
        nc.gpsimd.memset(mask_add[:], 0.0)
        nc.gpsimd.affine_select(
            out=mask_add[:],
            in_=mask_add[:],
            compare_op=mybir.AluOpType.is_ge,
            fill=-1e9,
            base=0,
            channel_multiplier=-1,
            pattern=[[1, P]],
        )
        # warm the exp table set off the critical path
        warm = const_pool.tile([1, 1], f32)
        nc.vector.memset(warm[:], 0.0)
        nc.scalar.activation(warm[:], warm[:],
                             mybir.ActivationFunctionType.Exp)

        pair_ctr = [0]

        for h in range(n_heads):
            qs_c, ks_c, vs_c = [], [], []
            for cch in range(n_ch):
                if h == 0 and cch == 0:
                    ks_c.append(k00)
                    qs_c.append(q00)
                    vs_c.append(v00)
                    continue
                kc = k_pool.tile([P, QC], bf16, tag="k")
                nc.sync.dma_start(out=kc[:], in_=kT[h, cch])
                ks_c.append(kc)
                qc_t = q_pool.tile([P, QC], bf16, tag="q")
                nc.gpsimd.dma_start(out=qc_t[:], in_=qT[h, cch])
                qs_c.append(qc_t)
                vc = v_pool.tile([P, QC // P, P], bf16, tag="v")
                (nc.gpsimd if cch % 2 else nc.sync).dma_start(
                    out=vc[:], in_=v[h, cch]
                )
                vs_c.append(vc)

            chunk_order = list(range(n_qc))
            if h == n_heads - 1:
                chunk_order = chunk_order[1:] + [0]  # short tail

            for qc in chunk_order:
                out_ps = o_psum.tile([P, QC], f32, tag="o")
                den_ps = d_psum.tile([P, QC], f32, tag="d")
                nkt = kt_per_qc * (qc + 1)
                n_below = nkt - kt_per_qc
                q_sl = qs_c[qc][:]
                # all of this chunk's P values (below + band), bf16
                pcb = pc_pool.tile([P, 16 * QC], bf16, tag="pcb")

                groups = []
                for t in range(n_below // 2):
                    off = pair_ctr[0] % OFF_MOD in OFF_PHASES
                    pair_ctr[0] += 1
                    groups.append(("below", t, [2 * t, 2 * t + 1], off))
                groups.append(("band", 0, [n_below, n_below + 1], False))
                groups.append(("band", 1, [n_below + 2, n_below + 3], False))

                def k_sl(kt):
                    return ks_c[kt // kt_per_qc][
                        :, (kt % kt_per_qc) * P:(kt % kt_per_qc + 1) * P]

                def emit_qk_exp(gi, qc=qc, groups=groups, q_sl=q_sl, pcb=pcb,
                                k_sl=k_sl):
                    kind, idx, gkts, off = groups[gi]
                    s_ps = s_psum.tile([P, 2 * QC], f32, tag="s")
                    po = gkts[0] * QC  # P destination offset (flat cols)
                    if kind == "below":
                        for i, kt in enumerate(gkts):
                            nc.tensor.matmul(
                                s_ps[:, i * QC:(i + 1) * QC],
                                lhsT=k_sl(kt), rhs=q_sl[:],
                                start=True, stop=True,
                            )
                        if off:
                            # Schraudolph: int16(round(x*A + B)) bits are the
                            # bf16 encoding of ~exp(x*SCALE)
                            nc.vector.tensor_scalar(
                                pcb[:, po:po + 2 * QC].bitcast(i16),
                                s_ps[:],
                                SCH_A, SCH_B,
                                mybir.AluOpType.mult, mybir.AluOpType.add,
                            )
                        else:
                            nc.scalar.activation(
                                pcb[:, po:po + 2 * QC], s_ps[:],
                                mybir.ActivationFunctionType.Exp,
                                scale=SCALE,
                            )
                    else:
                        for i, kt in enumerate(gkts):
                            c = kt * P - qc * QC
                            lo = max(c, 0)
                            nc.tensor.matmul(
                                s_ps[:, i * QC + lo:(i + 1) * QC],
                                lhsT=k_sl(kt), rhs=q_sl[:, lo:QC],
                                start=True, stop=True,
                            )
                        c0 = gkts[0] * P - qc * QC
                        # exp the raw scores (max ~exp(8.4)=4.4e3, bf16-safe);
                        # causal masking happens post-exp in SBUF on GpSimd
                        nc.scalar.activation(
                            pcb[:, po + c0:po + 2 * QC],
                            s_ps[:, c0:2 * QC],
                            mybir.ActivationFunctionType.Exp,
                            scale=SCALE,
                        )

                for gi in range(min(LA + 1, len(groups))):
                    emit_qk_exp(gi)

                first_pv = [True]
                first_den = [True]
                last_kt = nkt - 1
                n_pairs_below = n_below // 2
                # denominator plan for below pairs: merge adjacent pairs
                # (quads) via one extra VectorE add
                den_rhs = [None] * max(n_pairs_below, 1)

                def p_sl(kt, lo=0):
                    return pcb[:, kt * QC + lo:(kt + 1) * QC]

                for gi, (kind, idx, gkts, off) in enumerate(groups):
                    if kind == "below":
                        t = idx
                        # PV: two solo bf16 matmuls
                        for i, kt in enumerate(gkts):
                            nc.tensor.matmul(
                                out_ps[:],
                                lhsT=vs_c[kt // kt_per_qc][:, kt % kt_per_qc, :],
                                rhs=p_sl(kt),
                                start=first_pv[0], stop=False,
                            )
                            first_pv[0] = False
                        # den: every 2nd pair, reduce the quad (4 k-tiles)
                        # with two VectorE adds and one matmul. n_below is a
                        # multiple of 4 so quads always complete.
                        p01 = o_pool.tile([P, QC], bf16, tag="p01")
                        nc.vector.tensor_add(
                            p01[:], p_sl(gkts[0]), p_sl(gkts[1]))
                        den_rhs[t] = p01
                        if t % 2 == 1:
                            # quad: add two pair-sums, one den matmul / 4
                            # tiles (n_below is a multiple of 4)
                            p03 = o_pool.tile([P, QC], bf16, tag="p03")
                            nc.vector.tensor_add(
                                p03[:], den_rhs[t - 1][:], p01[:])
                            nc.tensor.matmul(
                                den_ps[:],
                                lhsT=ones_bf[:], rhs=p03[:],
                                start=first_den[0], stop=False,
                            )
                            first_den[0] = False
                    else:
                        c0 = gkts[0] * P - qc * QC
                        for i, kt in enumerate(gkts):
                            c = kt * P - qc * QC
                            blk = pcb[:, kt * QC + c:kt * QC + c + P]
                            # zero strictly-below-diagonal of the 128x128
                            # diagonal block post-exp (keep where col >= row)
                            nc.gpsimd.affine_select(
                                out=blk, in_=blk,
                                compare_op=mybir.AluOpType.is_ge,
                                fill=0.0, base=0,
                                channel_multiplier=-1,
                                pattern=[[1, P]],
                            )
                            if BAND_DEN_PAIR and i == 1:
                                # zero the second tile's exp'd garbage cols
                                # (below its shrink point) for clean pair sums
                                nc.gpsimd.memset(
                                    pcb[:, kt * QC + c0:kt * QC + c], 0.0)
                        # PV: solo bf16 matmuls with causal shrink
                        for i, kt in enumerate(gkts):
                            c = kt * P - qc * QC
                            nc.tensor.matmul(
                                out_ps[:, c:QC],
                                lhsT=vs_c[kt // kt_per_qc][:, kt % kt_per_qc, :],
                                rhs=p_sl(kt, c),
                                start=first_pv[0], stop=(kt == last_kt),
                            )
                            first_pv[0] = False
                        if BAND_DEN_PAIR:
                            # garbage cols were zeroed: pre-add the band pair
                            pb2 = o_pool.tile([P, QC], bf16, tag="p01")
                            nc.vector.tensor_add(
                                pb2[:, c0:QC],
                                p_sl(gkts[0], c0), p_sl(gkts[1], c0))
                            nc.tensor.matmul(
                                den_ps[:, c0:QC],
                                lhsT=ones_bf[:], rhs=pb2[:, c0:QC],
                                start=first_den[0],
                                stop=(gkts[1] == last_kt),
                            )
                            first_den[0] = False
                        else:
                            for i, kt in enumerate(gkts):
                                c = kt * P - qc * QC
                                nc.tensor.matmul(
                                    den_ps[:, c:QC],
                                    lhsT=ones_bf[:], rhs=p_sl(kt, c),
                                    start=first_den[0], stop=(kt == last_kt),
                                )
                                first_den[0] = False
                    if gi + LA + 1 < len(groups):
                        emit_qk_exp(gi + LA + 1)

                recip = o_pool.tile([P, QC], f32, tag="r")
                nc.vector.reciprocal_approx_fast(out=recip[:], in_=den_ps[:])
                o_sb = o_pool.tile([P, QC], bf16, tag="os")
                nc.vector.tensor_mul(o_sb[:], out_ps[:], recip[:])
                nc.sync.dma_start(out=outT[h, qc], in_=o_sb[:])

    nc.compile()
    return nc


def pack_shard(qh, kh, vh):
    """Pack per-core arrays [n_heads, s, D] into the kernel's DRAM layouts."""
    import ml_dtypes
    nh, s, _ = qh.shape
    n_ch = s // QC
    qT = np.ascontiguousarray(
        qh.transpose(0, 2, 1).reshape(nh, D, n_ch, QC).transpose(0, 2, 1, 3)
    ).astype(ml_dtypes.bfloat16)
    kT = np.ascontiguousarray(
        kh.transpose(0, 2, 1).reshape(nh, D, n_ch, QC).transpose(0, 2, 1, 3)
    ).astype(ml_dtypes.bfloat16)
    v5 = np.ascontiguousarray(
        vh.reshape(nh, n_ch, QC // P, P, D).transpose(0, 1, 3, 2, 4)
    ).astype(ml_dtypes.bfloat16)
    return {"qT": qT, "kT": kT, "v": v5}


def unpack_out(outT):
    """outT [nh, n_ch, D, QC] bf16 -> [nh, s, D] f32."""
    nh, n_ch, _, _ = outT.shape
    o = outT.astype(np.float32).transpose(0, 2, 1, 3).reshape(nh, D, n_ch * QC)
    return o.transpose(0, 2, 1)


_NC_CACHE = {}


def _get_module():
    key = (HEADS_PER_CORE, S)
    if key not in _NC_CACHE:
        _NC_CACHE[key] = build_module(*key)
    return _NC_CACHE[key]


def kernel(q, k, v):
    from concourse.bass_utils import run_bass_kernel_spmd

    q = np.asarray(q, dtype=np.float32)
    k = np.asarray(k, dtype=np.float32)
    v = np.asarray(v, dtype=np.float32)

    qf = q.reshape(B * H, S, D)
    kf = k.reshape(B * H, S, D)
    vf = v.reshape(B * H, S, D)
    hpc = HEADS_PER_CORE
    in_maps = [
        pack_shard(
            qf[c * hpc:(c + 1) * hpc],
            kf[c * hpc:(c + 1) * hpc],
            vf[c * hpc:(c + 1) * hpc],
        )
        for c in range(N_CORES)
    ]

    nc = _get_module()
    res = run_bass_kernel_spmd(nc, in_maps, core_ids=list(range(N_CORES)))
    out = np.concatenate(
        [unpack_out(r["outT"]) for r in res.results], axis=0
    ).reshape(B, H, S, D)
    return np.ascontiguousarray(out.astype(np.float32))
